# revision 20
# baseline (speedup 1.0000x reference)
"""MoE FFN (top-2, capacity 1280) Trainium2 kernel.

Shapes (hardcoded): x [2,2048,1024] f32, Wr [1024,8] f32,
W1 [8,1024,4096] f32, W2 [8,2048,1024] f32.
N=4096 tokens, H=1024, E=8 experts, F=2048, cap=1280.

Sharding: expert-parallel — core e owns expert e's FFN (weights in bf16).
The router (logits/softmax/top-2/capacity bookkeeping) is replicated on
every core in fp32 so routing decisions match the reference exactly.
Each core scatters its expert's gate-weighted FFN rows into its own
zero-initialised partial-y; the host unshards by summing the 8 partials
(each token receives exactly its <=2 expert contributions). The balance
loss is computed on device (identical on all cores; core 0's is returned).

Device pipeline per core:
  R:  token-major router matmuls (fp32)  -> logits LG [128,32,8]
  B:  batched softmax/top-2/one-hots (DVE, whole-batch tiles)
  C:  capacity cumsum: per-chunk triangular-ones matmul + K=1-matmul
      carry broadcast chain (exact reference token order)
  S:  per-chunk indirect-DMA scatter of (token_id, gate_w) into a DRAM
      slot table at this expert's capacity positions; read back -> slot
      tables (gather/scatter indices + per-slot weights)
  G:  indirect gather of x rows (cast f32->bf16) + PE transpose -> xeT
  F1: hT = W1^T @ xeT (bf16), SwiGLU -> actT (bf16)
  F2: out = actT^T @ W2 (bf16), scale rows by gate weight, indirect
      scatter into partial y (f32)
"""
import numpy as np
from contextlib import ExitStack

import ml_dtypes

import concourse.bass as bass
import concourse.tile as tile
from concourse.tile import add_dep_helper
from concourse import bacc, mybir
from concourse.bass_utils import run_bass_kernel_spmd
from concourse.masks import make_identity

P = 128
N = 4096          # tokens
H = 1024          # hidden
E = 8             # experts
F2 = 4096         # 2*F
F = 2048
CAP = 1280        # capacity per expert
NM = N // P       # 32 token chunks
KH = H // P       # 8 H-chunks
KF = F // P       # 16 F-chunks
NSLOT = CAP // P  # 10 slot chunks
BIG = 1.0e6       # invalid index marker
ALU = mybir.AluOpType
ACTF = mybir.ActivationFunctionType
AXX = mybir.AxisListType.X

_CACHED = {}


def _bc_inner(ap, k):
    """[P, M] -> [P, M, k] broadcast (stride-0 inner axis)."""
    return bass.AP(tensor=ap.tensor, offset=ap.offset,
                   ap=[ap.ap[0], ap.ap[1], [0, k]])


def _bc_mid(ap, m):
    """[P, K] -> [P, m, K] broadcast (stride-0 middle axis)."""
    return bass.AP(tensor=ap.tensor, offset=ap.offset,
                   ap=[ap.ap[0], [0, m], ap.ap[1]])


def _build(n_cores):
    dt = mybir.dt
    nc = bacc.Bacc("TRN2", target_bir_lowering=False, debug=False,
                   num_devices=n_cores)

    # ---------------- DRAM I/O ----------------
    xts = nc.dram_tensor("xts", [P, 4, KH, P], dt.float32,
                         kind="ExternalInput")
    #   xts[p, j, k, c] = x[(4*cid+j)*128+c, k*128+p]  (this core's shard)
    xf = nc.dram_tensor("xf", [N, H], dt.float32, kind="ExternalInput")
    wr = nc.dram_tensor("wr", [P, KH, E], dt.float32, kind="ExternalInput")
    w1 = nc.dram_tensor("w1", [KF // 4, P, 2, 4, KH, P], dt.float16,
                        kind="ExternalInput")
    #   w1[g, p, ab, j, k, c] = W1[e][k*128+p, (16*ab + 4g + j)*128 + c]
    w2 = nc.dram_tensor("w2", [P, KF, H], dt.float16, kind="ExternalInput")
    #   w2[p, k, h] = W2[e][k*128+p, h]
    iota8 = nc.dram_tensor("iota8", [P, E], dt.float32, kind="ExternalInput")
    iotarev = nc.dram_tensor("iotarev", [P, E], dt.float32, kind="ExternalInput")
    triu = nc.dram_tensor("triu", [P, P], dt.bfloat16, kind="ExternalInput")
    tokid = nc.dram_tensor("tokid", [P, NM], dt.float32, kind="ExternalInput")
    eid = nc.dram_tensor("eid", [P, 1], dt.float32, kind="ExternalInput")

    y = nc.dram_tensor("y", [N, H], dt.float32, kind="ExternalOutput")
    loss = nc.dram_tensor("loss", [1, 1], dt.float32, kind="ExternalOutput")

    slots = nc.dram_tensor("slots", [CAP, 2], dt.float32)  # internal scratch
    slots_v = slots[:, :].rearrange("(s p) c -> p s c", p=P)  # [128, 10, 2]
    agin = nc.dram_tensor("agin", [N // E, 12], dt.float32)
    agout = nc.dram_tensor("agout", [N, 12], dt.float32, addr_space="Shared")
    agin_v = agin[:, :].rearrange("(j p) f -> p j f", p=P)    # [128, 4, 12]
    agout_v = agout[:, :].rearrange("(m p) f -> p m f", p=P)  # [128, 32, 12]

    with ExitStack() as ctx:
        tc = ctx.enter_context(tile.TileContext(nc))
        const = ctx.enter_context(tc.tile_pool(name="const", bufs=1))
        persist = ctx.enter_context(tc.tile_pool(name="persist", bufs=1))
        work = ctx.enter_context(tc.tile_pool(name="work", bufs=3))
        small = ctx.enter_context(tc.tile_pool(name="small", bufs=2))

        # ---------------- constants ----------------
        iota8_t = const.tile([P, E], dt.float32)
        nc.sync.dma_start(iota8_t[:], iota8[:, :])
        iotarev_t = const.tile([P, E], dt.float32)
        nc.sync.dma_start(iotarev_t[:], iotarev[:, :])
        triu_t = const.tile([P, P], dt.bfloat16)
        nc.sync.dma_start(triu_t[:], triu[:, :])
        tokid_t = const.tile([P, NM], dt.float32)
        nc.sync.dma_start(tokid_t[:], tokid[:, :])
        eid_t = const.tile([P, 1], dt.float32)
        nc.sync.dma_start(eid_t[:], eid[:, :])
        wr_t = const.tile([P, KH, E], dt.float32)
        nc.sync.dma_start(wr_t[:], wr[:, :, :])
        id32 = const.tile([P, P], dt.float32)
        make_identity(nc, id32)
        id16 = const.tile([P, P], dt.float16)
        nc.vector.tensor_copy(id16[:], id32[:])
        ones_col16 = const.tile([P, 1], dt.bfloat16)
        nc.vector.memset(ones_col16[:], 1.0)
        ones_col32 = const.tile([P, 1], dt.float32)
        nc.vector.memset(ones_col32[:], 1.0)
        ones_row32 = const.tile([1, P], dt.float32)
        nc.vector.memset(ones_row32[:], 1.0)
        ones_sq16 = const.tile([P, P], dt.bfloat16)
        nc.vector.memset(ones_sq16[:], 1.0)

        # ---------------- batched router state ----------------
        OH0F = persist.tile([P, NM, E], dt.float32)
        OH1F = persist.tile([P, NM, E], dt.float32)
        OHB = persist.tile([P, NM, 2 * E], dt.bfloat16)
        PREF = persist.tile([P, NM, 2 * E], dt.float32)
        ROUT = persist.tile([P, NM, 12], dt.float32)

        ctx_acc = ExitStack()
        pacc = ctx_acc.enter_context(tc.tile_pool(name="pacc", bufs=1, space="PSUM"))
        ps_gate = pacc.tile([1, E], dt.float32, space="PSUM")
        ps_used1 = pacc.tile([1, E], dt.float32, space="PSUM")
        ps_tot = pacc.tile([1, 2 * E], dt.float32, space="PSUM")

        # zero the slot table (scatter skips empty slots; w==0 marks empty)
        zt = small.tile([P, NSLOT * 2], dt.float32, tag="zt")
        nc.vector.memset(zt[:], 0.0)
        nc.sync.dma_start(slots_v, zt[:].rearrange("p (s c) -> p s c", c=2))

        # ============ R: data-parallel router (this core's 4 chunks) ============
        NL = 4  # local chunks
        ctx_r = ExitStack()
        pr = ctx_r.enter_context(tc.tile_pool(name="pr", bufs=2, space="PSUM"))
        xtg = work.tile([P, NL, KH, P], dt.float32, tag="xtg", bufs=2)
        last_xt_dma = nc.sync.dma_start(xtg[:], xts[:, :, :, :])
        LGL = small.tile([P, NL, E], dt.float32, tag="LGL")
        for j in range(NL):
            ps_l = pr.tile([P, E], dt.float32, space="PSUM", tag="ps_l")
            for k in range(KH):
                nc.tensor.matmul(ps_l[:], xtg[:, j, k, :], wr_t[:, k, :],
                                 start=(k == 0), stop=(k == KH - 1))
            nc.vector.tensor_copy(LGL[:, j, :], ps_l[:])

        # local softmax + top-2 (logits are small: exp w/o max-shift is safe)
        EXL = small.tile([P, NL, E], dt.float32, tag="EXL")
        nc.scalar.activation(EXL[:], LGL[:], ACTF.Exp)
        S1L = small.tile([P, NL], dt.float32, tag="S1L")
        nc.vector.tensor_reduce(S1L[:], EXL[:], axis=AXX, op=ALU.add)
        RCL = small.tile([P, NL], dt.float32, tag="RCL")
        nc.vector.reciprocal(RCL[:], S1L[:])
        PK = small.tile([P, NL, 12], dt.float32, tag="PK")
        nc.vector.tensor_tensor(PK[:, :, 4:12], EXL[:], _bc_inner(RCL[:], E),
                                op=ALU.mult)

        M1L = small.tile([P, NL], dt.float32, tag="M1L")
        EQL = small.tile([P, NL, E], dt.float32, tag="EQL")
        RML = small.tile([P, NL], dt.float32, tag="RML")
        OHL = small.tile([P, NL, E], dt.float32, tag="OHL")
        nc.vector.tensor_reduce(M1L[:], LGL[:], axis=AXX, op=ALU.max)
        nc.vector.tensor_tensor(EQL[:], LGL[:], _bc_inner(M1L[:], E), op=ALU.is_equal)
        nc.vector.tensor_tensor(EQL[:], EQL[:], _bc_mid(iotarev_t[:], NL), op=ALU.mult)
        nc.vector.tensor_reduce(RML[:], EQL[:], axis=AXX, op=ALU.max)
        nc.vector.tensor_scalar(PK[:, :, 0], RML[:], -1.0, float(E - 1),
                                ALU.mult, ALU.add)
        nc.vector.tensor_tensor(OHL[:], _bc_mid(iota8_t[:], NL),
                                _bc_inner(PK[:, :, 0], E), op=ALU.is_equal)
        TMPL = small.tile([P, NL, E], dt.float32, tag="TMPL")
        nc.vector.tensor_tensor(TMPL[:], PK[:, :, 4:12], OHL[:], op=ALU.mult)
        nc.vector.tensor_reduce(PK[:, :, 2], TMPL[:], axis=AXX, op=ALU.add)

        LG2L = small.tile([P, NL, E], dt.float32, tag="LG2L")
        nc.vector.scalar_tensor_tensor(LG2L[:], OHL[:], -1.0e30, LGL[:],
                                       op0=ALU.mult, op1=ALU.add)
        nc.vector.tensor_reduce(M1L[:], LG2L[:], axis=AXX, op=ALU.max)
        nc.vector.tensor_tensor(EQL[:], LG2L[:], _bc_inner(M1L[:], E),
                                op=ALU.is_equal)
        nc.vector.tensor_tensor(EQL[:], EQL[:], _bc_mid(iotarev_t[:], NL),
                                op=ALU.mult)
        nc.vector.tensor_reduce(RML[:], EQL[:], axis=AXX, op=ALU.max)
        nc.vector.tensor_scalar(PK[:, :, 1], RML[:], -1.0, float(E - 1),
                                ALU.mult, ALU.add)
        nc.vector.tensor_tensor(OHL[:], _bc_mid(iota8_t[:], NL),
                                _bc_inner(PK[:, :, 1], E), op=ALU.is_equal)
        nc.vector.tensor_tensor(TMPL[:], PK[:, :, 4:12], OHL[:], op=ALU.mult)
        nc.vector.tensor_reduce(PK[:, :, 3], TMPL[:], axis=AXX, op=ALU.add)

        # all-gather the routing info (e0, e1, v0, v1, gate[8]) for all tokens
        nc.sync.dma_start(agin_v, PK[:])
        nc.gpsimd.collective_compute(
            "AllGather", ALU.bypass, replica_groups=[list(range(E))],
            ins=[agin[:, :]], outs=[agout[:, :]])
        nc.sync.dma_start(ROUT[:], agout_v)

        E0 = ROUT[:, :, 0]
        E1 = ROUT[:, :, 1]
        V0 = ROUT[:, :, 2]
        V1 = ROUT[:, :, 3]
        nc.vector.tensor_tensor(OH0F[:], _bc_mid(iota8_t[:], NM), _bc_inner(E0, E),
                                op=ALU.is_equal)
        nc.vector.tensor_tensor(OH1F[:], _bc_mid(iota8_t[:], NM), _bc_inner(E1, E),
                                op=ALU.is_equal)
        nc.vector.tensor_copy(OHB[:, :, :E], OH0F[:])
        nc.vector.tensor_copy(OHB[:, :, E:], OH1F[:])

        # gate-mean for balance loss
        for m in range(NM):
            nc.tensor.matmul(ps_gate[:], ones_col32[:, :1], ROUT[:, m, 4:12],
                             start=(m == 0), stop=(m == NM - 1))

        # ============ C: capacity cumsum, 2 chunks per carry step ============
        for h in range(NM // 2):
            m = 2 * h
            carry2 = small.tile([1, 2, 2 * E], dt.float32, tag="carry2")
            if h == 0:
                nc.vector.memset(carry2[:], 0.0)
            else:
                nc.vector.tensor_copy(carry2[:], _bc_mid(ps_tot[:1, :], 2))
            ps_pref = pr.tile([P, 2, 2 * E], dt.float32, space="PSUM",
                              tag="ps_pref")
            # within-chunk inclusive cumsum for both chunks
            nc.tensor.matmul(ps_pref[:], triu_t[:], OHB[:, m:m + 2, :],
                             start=True, stop=False)
            # chunk m's column totals flow into chunk m+1
            nc.tensor.matmul(ps_pref[:, 1, :], ones_sq16[:], OHB[:, m, :],
                             start=False, stop=False)
            # running carry into both chunks
            nc.tensor.matmul(ps_pref[:].rearrange("p a b -> p (a b)"),
                             ones_row32[:1, :],
                             carry2[:].rearrange("p a b -> p (a b)"),
                             start=False, stop=True)
            nc.vector.tensor_copy(PREF[:, m:m + 2, :], ps_pref[:])
            nc.tensor.matmul(ps_tot[:], ones_col16[:, :1], OHB[:, m, :],
                             start=(h == 0), stop=False)
            nc.tensor.matmul(ps_tot[:], ones_col16[:, :1], OHB[:, m + 1, :],
                             start=False, stop=(h == NM // 2 - 1))

        # used0 = min(count0, cap), broadcast to all partitions
        used0 = small.tile([1, E], dt.float32, tag="used0")
        nc.vector.tensor_scalar(used0[:], ps_tot[:1, :E], float(CAP), None, ALU.min)
        ps_b = pr.tile([P, E], dt.float32, space="PSUM", tag="ps_l", name="ps_b")
        nc.tensor.matmul(ps_b[:], ones_row32[:1, :], used0[:], start=True, stop=True)
        used0b = small.tile([P, E], dt.float32, tag="used0b")
        nc.vector.tensor_copy(used0b[:], ps_b[:])

        # ============ batched keeps / positions / scatter payload ============
        TMP2 = small.tile([P, NM, E], dt.float32, tag="TMP2")
        P0 = small.tile([P, NM], dt.float32, tag="P0")
        nc.vector.tensor_tensor(TMP2[:], PREF[:, :, :E], OH0F[:], op=ALU.mult)
        nc.vector.tensor_reduce(P0[:], TMP2[:], axis=AXX, op=ALU.add)
        P0EX = small.tile([P, NM], dt.float32, tag="P0EX")
        nc.vector.tensor_scalar(P0EX[:], P0[:], 1.0, None, ALU.subtract)
        KEEP0 = small.tile([P, NM], dt.float32, tag="KEEP0")
        nc.vector.tensor_scalar(KEEP0[:], P0EX[:], float(CAP), None, ALU.is_lt)

        U0E1 = small.tile([P, NM], dt.float32, tag="U0E1")
        nc.vector.tensor_tensor(TMP2[:], OH1F[:], _bc_mid(used0b[:], NM), op=ALU.mult)
        nc.vector.tensor_reduce(U0E1[:], TMP2[:], axis=AXX, op=ALU.add)
        P1R = small.tile([P, NM], dt.float32, tag="P1R")
        nc.vector.tensor_tensor(TMP2[:], PREF[:, :, E:], OH1F[:], op=ALU.mult)
        nc.vector.tensor_reduce(P1R[:], TMP2[:], axis=AXX, op=ALU.add)
        P1EX = small.tile([P, NM], dt.float32, tag="P1EX")
        nc.vector.scalar_tensor_tensor(P1EX[:], P1R[:], -1.0, U0E1[:],
                                       op0=ALU.add, op1=ALU.add)
        KEEP1 = small.tile([P, NM], dt.float32, tag="KEEP1")
        nc.vector.tensor_scalar(KEEP1[:], P1EX[:], float(CAP), None, ALU.is_lt)

        # used1 accumulation (for balance loss)
        nc.vector.tensor_tensor(TMP2[:], OH1F[:], _bc_inner(KEEP1[:], E), op=ALU.mult)
        for m in range(NM):
            nc.tensor.matmul(ps_used1[:], ones_col32[:, :1], TMP2[:, m, :],
                             start=(m == 0), stop=(m == NM - 1))

        # my-expert selection
        SEL0 = small.tile([P, NM], dt.float32, tag="SEL0")
        nc.vector.tensor_scalar(SEL0[:], E0, eid_t[:, :1], None, ALU.is_equal)
        nc.vector.tensor_tensor(SEL0[:], SEL0[:], KEEP0[:], op=ALU.mult)
        SEL1 = small.tile([P, NM], dt.float32, tag="SEL1")
        nc.vector.tensor_scalar(SEL1[:], E1, eid_t[:, :1], None, ALU.is_equal)
        nc.vector.tensor_tensor(SEL1[:], SEL1[:], KEEP1[:], op=ALU.mult)

        # posA = sel0*(p0ex+1) + sel1*(p1ex+1) - 1 ; non-mine -> +BIG
        T1 = small.tile([P, NM], dt.float32, tag="T1")
        nc.vector.scalar_tensor_tensor(T1[:], P0EX[:], 1.0, SEL0[:],
                                       op0=ALU.add, op1=ALU.mult)
        T2 = small.tile([P, NM], dt.float32, tag="T2")
        nc.vector.scalar_tensor_tensor(T2[:], P1EX[:], 1.0, SEL1[:],
                                       op0=ALU.add, op1=ALU.mult)
        POSA = small.tile([P, NM], dt.float32, tag="POSA")
        nc.vector.scalar_tensor_tensor(POSA[:], T1[:], -1.0 + BIG, T2[:],
                                       op0=ALU.add, op1=ALU.add)
        SELN = small.tile([P, NM], dt.float32, tag="SELN")
        nc.vector.tensor_tensor(SELN[:], SEL0[:], SEL1[:], op=ALU.add)
        nc.vector.scalar_tensor_tensor(POSA[:], SELN[:], -BIG, POSA[:],
                                       op0=ALU.mult, op1=ALU.add)
        POSI = persist.tile([P, NM], dt.int32)
        nc.vector.tensor_copy(POSI[:], POSA[:])

        # scatter payload SC[:, m, :] = (token_id, gate_w)
        WSEL = small.tile([P, NM], dt.float32, tag="WSEL")
        nc.vector.tensor_tensor(T1[:], SEL0[:], V0, op=ALU.mult)
        nc.vector.tensor_tensor(T2[:], SEL1[:], V1, op=ALU.mult)
        nc.vector.tensor_tensor(WSEL[:], T1[:], T2[:], op=ALU.add)
        SC = persist.tile([P, NM, 2], dt.float32)
        nc.vector.tensor_copy(SC[:, :, 0], tokid_t[:])
        nc.vector.tensor_copy(SC[:, :, 1], WSEL[:])

        # ============ balance loss ============
        prob = small.tile([1, E], dt.float32, tag="prob")
        nc.vector.tensor_copy(prob[:], ps_gate[:1, :])
        used = small.tile([1, E], dt.float32, tag="used")
        nc.vector.tensor_tensor(used[:], used0[:], ps_used1[:1, :], op=ALU.add)
        frac = small.tile([1, E], dt.float32, tag="frac")
        me = 1e-9
        nc.vector.tensor_scalar(frac[:], used[:], me, 1.0 / (2.0 * N + me),
                                ALU.max, ALU.mult)
        nc.vector.tensor_tensor(frac[:], prob[:], frac[:], op=ALU.mult)
        lsum = small.tile([1, 1], dt.float32, tag="lsum")
        nc.vector.tensor_reduce(lsum[:], frac[:], axis=AXX, op=ALU.add)
        lout = small.tile([1, 1], dt.float32, tag="lout")
        nc.vector.tensor_scalar(lout[:], lsum[:], float(E) / float(N), None, ALU.mult)
        nc.sync.dma_start(loss[:, :], lout[:])
        ctx_r.close()
        ctx_acc.close()

        # ============ S: scatter slot table + read back ============
        for m in range(NM):
            nc.gpsimd.indirect_dma_start(
                out=slots[:, :],
                out_offset=bass.IndirectOffsetOnAxis(ap=POSI[:, m:m + 1], axis=0),
                in_=SC[:, m, :], in_offset=None,
                bounds_check=CAP - 1, oob_is_err=False)
        SL = persist.tile([P, NSLOT, 2], dt.float32)
        nc.sync.dma_start(SL[:], slots_v)
        OCC = small.tile([P, NSLOT], dt.float32, tag="OCC")
        nc.vector.tensor_scalar(OCC[:], SL[:, :, 1], 0.0, None, ALU.is_gt)
        GF = small.tile([P, NSLOT], dt.float32, tag="GF")
        nc.vector.scalar_tensor_tensor(GF[:], SL[:, :, 0], -BIG, OCC[:],
                                       op0=ALU.add, op1=ALU.mult)
        nc.vector.tensor_scalar(GF[:], GF[:], BIG, None, ALU.add)
        GIDX = persist.tile([P, NSLOT], dt.int32)
        nc.vector.tensor_copy(GIDX[:], GF[:])

        # ============ G: gather xe + transpose to xeT ============
        ctx_t = ExitStack()
        ptp = ctx_t.enter_context(tc.tile_pool(name="ptp", bufs=3, space="PSUM"))
        xsz = [512, 512, 256]
        xet = [[persist.tile([P, xsz[jj]], dt.float16, tag=f"xet{k}_{jj}",
                             name=f"xet{k}_{jj}") for jj in range(3)]
               for k in range(KH)]
        for s in range(NSLOT):
            jj, scol = (0, s) if s < 4 else ((1, s - 4) if s < 8 else (2, s - 8))
            xe = work.tile([P, H], dt.float16, tag="xe")
            nc.gpsimd.indirect_dma_start(
                out=xe[:], out_offset=None, in_=xf[:, :],
                in_offset=bass.IndirectOffsetOnAxis(ap=GIDX[:, s:s + 1], axis=0),
                bounds_check=N - 1, oob_is_err=False)
            for k in range(KH):
                ps_x = ptp.tile([P, P], dt.float16, space="PSUM",
                                tag="ps_x", name="ps_x")
                nc.tensor.transpose(ps_x[:], xe[:, k * P:(k + 1) * P], id16[:])
                eng = nc.vector if (k % 2 == 0) else nc.scalar
                if eng is nc.scalar:
                    nc.scalar.copy(xet[k][jj][:, scol * P:(scol + 1) * P], ps_x[:])
                else:
                    nc.vector.tensor_copy(xet[k][jj][:, scol * P:(scol + 1) * P],
                                          ps_x[:])
        ctx_t.close()

        # preload W2 into SBUF
        w2_sb = persist.tile([P, KF, H], dt.float16)
        w2_dma = nc.sync.dma_start(w2_sb[:], w2[:, :, :])
        add_dep_helper(w2_dma.ins, last_xt_dma.ins, sync=False,
                       reason="delay weight load until router input is read")

        # ============ F1: hT = W1^T @ xeT, SwiGLU -> actT ============
        actt = [persist.tile([P, CAP], dt.float16, tag=f"actt{k}", name=f"actt{k}")
                for k in range(KF)]
        ctx_f = ExitStack()
        pff = ctx_f.enter_context(tc.tile_pool(name="pff", bufs=2, space="PSUM"))
        for g in range(KF // 4):
            w1g = work.tile([P, 2, 4, KH, P], dt.float16, tag="w1g", bufs=2)
            w1_dma = nc.sync.dma_start(w1g[:], w1[g, :, :, :, :, :])
            add_dep_helper(w1_dma.ins, last_xt_dma.ins, sync=False,
                           reason="delay weight load until router input is read")
            for j in range(4):
                mp = 4 * g + j
                for jj in range(3):
                    nsz = xsz[jj]
                    lo = jj * 512
                    ps_a = pff.tile([P, 512], dt.float32, space="PSUM", tag="ps_a")
                    ps_bb = pff.tile([P, 512], dt.float32, space="PSUM", tag="ps_bb")
                    for k in range(KH):
                        nc.tensor.matmul(ps_a[:, :nsz], w1g[:, 0, j, k, :],
                                         xet[k][jj][:], start=(k == 0),
                                         stop=(k == KH - 1))
                    for k in range(KH):
                        nc.tensor.matmul(ps_bb[:, :nsz], w1g[:, 1, j, k, :],
                                         xet[k][jj][:], start=(k == 0),
                                         stop=(k == KH - 1))
                    sa = work.tile([P, 512], dt.float32, tag="sa")
                    nc.scalar.activation(sa[:, :nsz], ps_a[:, :nsz], ACTF.Silu)
                    nc.vector.tensor_tensor(actt[mp][:, lo:lo + nsz], sa[:, :nsz],
                                            ps_bb[:, :nsz], op=ALU.mult)

        # ============ F2: out = actT^T @ W2, scale, scatter ============
        for mo in range(NSLOT):
            outp = work.tile([P, H], dt.float32, tag="outp")
            for hb in range(2):
                ps_o = pff.tile([P, 512], dt.float32, space="PSUM", tag="ps_o")
                for k in range(KF):
                    nc.tensor.matmul(ps_o[:], actt[k][:, mo * P:(mo + 1) * P],
                                     w2_sb[:, k, hb * 512:(hb + 1) * 512],
                                     start=(k == 0), stop=(k == KF - 1))
                nc.vector.tensor_scalar(outp[:, hb * 512:(hb + 1) * 512], ps_o[:],
                                        SL[:, mo, 1:2], None, ALU.mult)
            nc.gpsimd.indirect_dma_start(
                out=y[:, :],
                out_offset=bass.IndirectOffsetOnAxis(ap=GIDX[:, mo:mo + 1], axis=0),
                in_=outp[:], in_offset=None,
                bounds_check=N - 1, oob_is_err=False)
        ctx_f.close()

    nc.finalize()
    return nc


def _prep_inputs(x, Wr, W1, W2):
    bf16 = ml_dtypes.bfloat16
    xflat = np.ascontiguousarray(x.reshape(N, H).astype(np.float32))
    xt = np.ascontiguousarray(
        xflat.reshape(NM // 4, 4, P, KH, P).transpose(0, 4, 1, 3, 2))
    # per-core router shard
    wr = np.ascontiguousarray(
        Wr.astype(np.float32).reshape(KH, P, E).transpose(1, 0, 2))
    iota8 = np.broadcast_to(np.arange(E, dtype=np.float32), (P, E)).copy()
    iotarev = np.broadcast_to((E - 1) - np.arange(E, dtype=np.float32), (P, E)).copy()
    triu = np.triu(np.ones((P, P), dtype=np.float32)).astype(bf16)
    tokid = (np.arange(NM, dtype=np.float32)[None, :] * P
             + np.arange(P, dtype=np.float32)[:, None]).astype(np.float32)

    common = dict(xf=xflat, wr=wr, iota8=iota8, iotarev=iotarev,
                  triu=triu, tokid=tokid)
    in_maps = []
    for e in range(E):
        w1e = W1[e].astype(np.float16)   # [H, 2F]
        # [KH, P(p), 2(ab), KF//4(g), 4(j), P(c)] -> [g, p, ab, j, k, c]
        w1t = np.ascontiguousarray(
            w1e.reshape(KH, P, 2, KF // 4, 4, P).transpose(3, 1, 2, 4, 0, 5))
        w2e = W2[e].astype(np.float16)   # [F, H]
        w2t = np.ascontiguousarray(w2e.reshape(KF, P, H).transpose(1, 0, 2))
        eidv = np.full((P, 1), float(e), dtype=np.float32)
        in_maps.append(dict(common, w1=w1t, w2=w2t, eid=eidv,
                            xts=np.ascontiguousarray(xt[e])))
    return in_maps


def kernel(x, Wr, W1, W2, _trace=False, _trace_kwargs=None):
    x = np.asarray(x, dtype=np.float32)
    Wr = np.asarray(Wr, dtype=np.float32)
    W1 = np.asarray(W1, dtype=np.float32)
    W2 = np.asarray(W2, dtype=np.float32)

    if "nc" not in _CACHED:
        _CACHED["nc"] = _build(E)
    nc = _CACHED["nc"]

    in_maps = _prep_inputs(x, Wr, W1, W2)
    kw = {}
    if _trace:
        kw = dict(trace=True, trace_kwargs=_trace_kwargs or {})
    res = run_bass_kernel_spmd(nc, in_maps, core_ids=list(range(E)), **kw)

    y = np.zeros((N, H), dtype=np.float32)
    for e in range(E):
        y += res.results[e]["y"]
    loss = np.float32(res.results[0]["loss"][0, 0])
    kernel._last_exec_time_ns = res.exec_time_ns
    return y.reshape(2, 2048, H), loss
